# revision 95
# baseline (speedup 1.0000x reference)
"""Masked multi-head attention on 8 trn2 NeuronCores (Bass/Tile).

Problem: B=1, N=4096, C=256, H=8 heads (Dh=32), dense 0/1 mask adj
shared across heads.  reference:
    qkv = x @ w_qkv.T ; q,k,v per head
    attn = softmax(where(adj>0, q@k.T*scale, -9e15))
    out  = (attn @ v) @ w_proj.T + b_proj

Sharding: sequence-parallel over query rows.  Core i handles query rows
[512*i, 512*(i+1)) for ALL heads; k/v are recomputed on every core from
the (small) full x.  The big tensor -- the 64MB mask -- is split 8 ways
with no replication and there are no collectives.

Per-core pipeline (bf16 matmuls, f32 PSUM), proven baseline structure
plus three changes:
  * startup DMAs are spread across the SP and Activation HWDGE trigger
    queues (+ Pool SWDGE for bulk) so the first matmul inputs land ~6us
    earlier;
  * the post-exp mask multiply runs as scalar_tensor_tensor
    ((p * 1.0) * mask) instead of tensor_tensor -- InstTensorScalarPtr
    is eligible for the DVE 4x_2p perf mode (tensor_tensor mult caps at
    2x_1p) -- with a separate output tile so all views stay SBUF/bf16;
  * V_TILES k-tiles skip ScalarE entirely: one fused DVE
    scalar_tensor_tensor computes int16 bf16-bits =
    round(s*(128/ln2) + maskbias), a Schraudolph exp with the additive
    mask folded into the int16 bias tile (same bytes as the bf16 mask,
    per-tile format).  Masked lanes get bits ~2^-43.
"""

import sys

for _p in ("/opt/trn_rl_repo", "/root/.axon_site/_ro/trn_rl_repo"):
    if _p not in sys.path:
        sys.path.insert(0, _p)

import numpy as np
import ml_dtypes

BF16NP = ml_dtypes.bfloat16

N = 4096
C = 256
H = 8
DH = 32
NCORES = 8
NQ = N // NCORES  # 512 query rows per core
KT = N // 128  # 32 key tiles

A2 = 128.0 / np.log(2.0)  # Schraudolph scale
B2I = 16249  # 16256 + C2, C2 = -7 (round-to-nearest calibrated)
MASK_DELTA = 5540  # round(30 * A2): masked lanes -> bits ~ 2^-43

# k-tiles whose exp runs as the fused DVE Schraudolph op (int16 mask bias).
# Empty: every attempt to offload exp off ScalarE lost time -- the score
# PSUM banks must be freed by ScalarE, not via the busy in-order DVE queue.
V_TILES = frozenset()

_CACHE = {}


def build_kernel():
    import concourse.bacc as bacc
    import concourse.tile as tile
    from concourse import mybir

    F32 = mybir.dt.float32
    BF = mybir.dt.bfloat16
    I16 = mybir.dt.int16
    EXP = mybir.ActivationFunctionType.Exp
    IDENT = mybir.ActivationFunctionType.Identity

    nc = bacc.Bacc("TRN2", target_bir_lowering=False, debug=False, num_devices=NCORES)

    xT_d = nc.dram_tensor("xT", [C, N], BF, kind="ExternalInput")
    xqT_d = nc.dram_tensor("xqT", [C, NQ], BF, kind="ExternalInput")
    wqkv_d = nc.dram_tensor("wqkv", [C, 3 * C], BF, kind="ExternalInput")
    wproj2_d = nc.dram_tensor("wproj2", [4 * 128, C], BF, kind="ExternalInput")
    bias2_d = nc.dram_tensor("bias2", [128, 2], F32, kind="ExternalInput")
    maskT_d = nc.dram_tensor("maskT", [N, 2 * NQ], I16, kind="ExternalInput")
    out_d = nc.dram_tensor("out", [C, NQ], F32, kind="ExternalOutput")

    with (
        tile.TileContext(nc) as tc,
        tc.tile_pool(name="consts", bufs=1) as consts,
        tc.tile_pool(name="ps", bufs=3, space="PSUM") as ps_pool,
        tc.tile_pool(name="pvps", bufs=2, space="PSUM") as pv_pool,
        tc.tile_pool(name="ptile", bufs=10) as p_pool,
        tc.tile_pool(name="dramp", bufs=1, space="DRAM") as dram_pool,
    ):
        # ---------------- input DMAs, spread across trigger queues ------
        w_sb = [
            consts.tile([128, 3 * C], BF, name=f"w_sb{c}", tag=f"w{c}")
            for c in range(2)
        ]
        xq_sb = [
            consts.tile([128, NQ], BF, name=f"xq_sb{c}", tag=f"xq{c}") for c in range(2)
        ]
        xT_sb = [
            consts.tile([128, N], BF, name=f"xT_sb{c}", tag=f"xT{c}") for c in range(2)
        ]
        mask_sb = consts.tile([128, KT, 2 * NQ], I16)
        mask_bf = mask_sb.bitcast(BF)
        maskT_r = maskT_d.rearrange("(m p) q -> p m q", p=128)

        nc.sync.dma_start(out=w_sb[0], in_=wqkv_d[0:128, :])
        nc.scalar.dma_start(out=w_sb[1], in_=wqkv_d[128:256, :])
        nc.scalar.dma_start(out=xq_sb[0], in_=xqT_d[0:128, :])
        nc.sync.dma_start(out=xq_sb[1], in_=xqT_d[128:256, :])
        nc.scalar.dma_start(out=xT_sb[0][:, 0:1024], in_=xT_d[0:128, 0:1024])
        nc.sync.dma_start(out=xT_sb[1][:, 0:1024], in_=xT_d[128:256, 0:1024])
        nc.sync.dma_start(out=xT_sb[0][:, 1024:2048], in_=xT_d[0:128, 1024:2048])
        nc.scalar.dma_start(out=xT_sb[1][:, 1024:2048], in_=xT_d[128:256, 1024:2048])
        nc.sync.dma_start(out=mask_sb[:, 0:2, :], in_=maskT_r[:, 0:2, :])
        nc.scalar.dma_start(out=mask_sb[:, 2:4, :], in_=maskT_r[:, 2:4, :])
        nc.gpsimd.dma_start(out=mask_sb[:, 4:6, :], in_=maskT_r[:, 4:6, :])
        # bulk tail + end-of-kernel tensors on the SWDGE path.  Mask tiles
        # 6:12 go BEFORE the xT tail: the mask-muls consume tiles at the
        # exp cadence from ~t+20us and a late chunk stalls the whole
        # exp->mul->pv chain once the probs pool fills.
        nc.gpsimd.dma_start(out=mask_sb[:, 6:12, :], in_=maskT_r[:, 6:12, :])
        # mask 12:16 also beats the xT tails: its muls fire at ~t+30us and
        # the 2MB of tails would push its arrival right onto that deadline
        nc.gpsimd.dma_start(out=mask_sb[:, 12:16, :], in_=maskT_r[:, 12:16, :])
        for ch in range(2, 4):
            cs = slice(ch * 1024, (ch + 1) * 1024)
            for c in range(2):
                nc.gpsimd.dma_start(
                    out=xT_sb[c][:, cs], in_=xT_d[128 * c : 128 * (c + 1), cs]
                )
        nc.gpsimd.dma_start(out=mask_sb[:, 16:22, :], in_=maskT_r[:, 16:22, :])
        nc.gpsimd.dma_start(out=mask_sb[:, 22:32, :], in_=maskT_r[:, 22:32, :])
        wp_sb = consts.tile([128, 4, C], BF)
        nc.gpsimd.dma_start(
            out=wp_sb, in_=wproj2_d.rearrange("(g p) c -> p g c", p=128)
        )
        bias_sb = consts.tile([128, 2], F32)
        nc.gpsimd.dma_start(out=bias_sb, in_=bias2_d[:])

        # ones-pattern lhsT for the last pair's denominator broadcast matmul
        # (out rows 0:64 <- rhs row 32, 64:128 <- row 96); memset FIRST so
        # it doubles as the warmup operand below.
        ones_bf = consts.tile([128, 128], BF)
        nc.vector.memset(ones_bf, 0.0)
        nc.vector.memset(ones_bf[32:33, 0:64], 1.0)
        nc.vector.memset(ones_bf[96:97, 64:128], 1.0)

        # PE p-state warmup: the clock needs ~3us of continuous execution
        # to reach 2.4GHz, and phase-1 otherwise runs at the 1.2GHz mid
        # state.  Burn dummy matmuls on the (early-memset) ones tile while
        # the input DMAs are still in flight.
        warm_ps = ps_pool.tile([128, 1024], F32, name="warm_ps", tag="s")

        def warm(n):
            for _ in range(n):
                nc.tensor.matmul(
                    out=warm_ps[:, 0:128],
                    lhsT=ones_bf,
                    rhs=ones_bf,
                    start=True,
                    stop=True,
                )

        warm(52)  # ramp + bridge the PE to the w-DMA arrival (~8.5us)

        # ---------------- phase 1: qT, kT, v ----------------
        qT_sb = [
            consts.tile([128, NQ], BF, name=f"qT_sb{g}", tag=f"qT{g}") for g in range(2)
        ]
        for g in range(2):
            q_ps = ps_pool.tile([128, 1024], F32, name="q_ps", tag="s")
            for c in range(2):
                nc.tensor.matmul(
                    out=q_ps[:, 0:NQ],
                    lhsT=w_sb[c][:, 128 * g : 128 * (g + 1)],
                    rhs=xq_sb[c],
                    start=(c == 0),
                    stop=(c == 1),
                )
            nc.vector.tensor_copy(out=qT_sb[g], in_=q_ps[:, 0:NQ])

        # keep the clock hot while the PE waits for the xT chunks: these
        # run in-order between q-proj and the first kT projections
        warm(14)

        kT_sb = [
            consts.tile([128, N], BF, name=f"kT_sb{g}", tag=f"kT{g}") for g in range(2)
        ]

        def emit_kT(g, n):
            k_ps = ps_pool.tile([128, 1024], F32, name="k_ps", tag="s")
            for c in range(2):
                nc.tensor.matmul(
                    out=k_ps[:, 0:512],
                    lhsT=w_sb[c][:, 256 + 128 * g : 256 + 128 * (g + 1)],
                    rhs=xT_sb[c][:, 512 * n : 512 * (n + 1)],
                    start=(c == 0),
                    stop=(c == 1),
                )
            nc.vector.tensor_copy(
                out=kT_sb[g][:, 512 * n : 512 * (n + 1)], in_=k_ps[:, 0:512]
            )

        v_sb = [
            consts.tile([128, 34 * H], BF, name=f"v_sb_{m}", tag=f"v{m}")
            for m in range(KT)
        ]

        def emit_v(m):
            v_ps = ps_pool.tile([128, 1024], F32, name="v_ps", tag="s")
            for c in range(2):
                nc.tensor.matmul(
                    out=v_ps[:, 0:C],
                    lhsT=xT_sb[c][:, 128 * m : 128 * (m + 1)],
                    rhs=w_sb[c][:, 512:768],
                    start=(c == 0),
                    stop=(c == 1),
                )
            vt3 = v_sb[m].rearrange("p (h w) -> p h w", h=H)
            nc.gpsimd.memset(vt3[:, :, 32:34], 1.0)
            nc.vector.tensor_copy(
                out=vt3[:, :, 0:32],
                in_=v_ps[:, 0:C].rearrange("p (h w) -> p h w", h=H),
            )

        for n in range(min(4, N // 512)):
            if n == 2:
                # bridge the wait for the second xT kilochunk so the kT
                # projections stay at full clock
                warm(10)
            emit_kT(0, n)

        # ---------------- phase 2: attention ----------------
        rec_sb = consts.tile([128, 4 * 512], F32)
        bc_cat = consts.tile([128, 4 * 512], F32)
        nc.vector.memset(bc_cat, 0.0)
        od_bf = consts.tile([128, NQ], BF)
        o_cat = [
            consts.tile([128, NQ], BF, name=f"o_cat{g2}", tag=f"oc{g2}")
            for g2 in range(4)
        ]
        o_raw = [
            consts.tile([128, NQ], F32, name=f"o_raw{g2}", tag=f"or{g2}")
            for g2 in range(4)
        ]

        nrm_scr = dram_pool.tile([4, 2, NQ], F32)

        def emit_norm(j):
            # broadcast reciprocal rows 32/96 across each head's 32
            # partitions via a DRAM round-trip: DMA the row out, DMA it back
            # with a partition-broadcast source AP (no PE/PSUM involved).
            import concourse.bass as bass

            for half, prow, orow in ((0, 32, 0), (1, 96, 64)):
                nc.sync.dma_start(
                    out=nrm_scr[j, half, :],
                    in_=rec_sb[prow : prow + 1, 512 * j : 512 * j + NQ],
                )
                row = nrm_scr[j, half : half + 1, :]
                bcast = bass.AP(
                    tensor=row.tensor,
                    offset=row.offset,
                    ap=[[0, 32]] + list(row.ap[1:]),
                )
                nc.sync.dma_start(
                    out=bc_cat[orow : orow + 32, 512 * j : 512 * j + NQ],
                    in_=bcast,
                )
            nc.vector.tensor_mul(
                out=o_cat[j],
                in0=o_raw[j],
                in1=bc_cat[:, 512 * j : 512 * j + NQ],
            )

        for g2 in range(4):  # head pairs
            hA, hB = 2 * g2, 2 * g2 + 1
            gA, pA = hA // 4, 32 * (hA % 4)
            gB, pB = hB // 4, 32 * (hB % 4)
            pv_t = pv_pool.tile([128, 512], F32, name="pv_t", tag="pv")
            if g2 < 2 or KT < 32:
                nc.vector.memset(pv_t, 1.0)
            for m in range(KT):
                # v is needed by this very tile's pv -> emit first
                if g2 == 0:
                    emit_v(m)
                s_ps = ps_pool.tile([128, 1024], F32, name="s_ps", tag="s")
                nc.tensor.matmul(
                    out=s_ps[:, 0:NQ],
                    lhsT=kT_sb[gA][pA : pA + 32, 128 * m : 128 * (m + 1)],
                    rhs=qT_sb[gA][pA : pA + 32, :],
                    start=True,
                    stop=True,
                    tile_position=(pA, 0),
                )
                nc.tensor.matmul(
                    out=s_ps[:, 512 : 512 + NQ],
                    lhsT=kT_sb[gB][pB : pB + 32, 128 * m : 128 * (m + 1)],
                    rhs=qT_sb[gB][pB : pB + 32, :],
                    start=True,
                    stop=True,
                    tile_position=(pB, 0),
                )
                p_sb = p_pool.tile([128, 2 * NQ], BF, name="p_sb", tag="p")
                if m in V_TILES:
                    # fused Schraudolph exp + mask in ONE DVE op: int16
                    # bf16-bits = round(s * A2 + maskbias)
                    nc.vector.scalar_tensor_tensor(
                        out=p_sb.bitcast(I16),
                        in0=s_ps,
                        scalar=float(A2),
                        in1=mask_sb[:, m, :],
                        op0=mybir.AluOpType.mult,
                        op1=mybir.AluOpType.add,
                    )
                else:
                    nc.scalar.activation(out=p_sb, in_=s_ps, func=EXP)
                    nc.vector.tensor_mul(out=p_sb, in0=p_sb, in1=mask_bf[:, m, :])
                first, last = (m == 0), (m == KT - 1)
                nc.tensor.matmul(
                    out=pv_t[0:33, 0:NQ],
                    lhsT=v_sb[m][:, 34 * hA : 34 * hA + 33],
                    rhs=p_sb[:, 0:NQ],
                    start=first,
                    stop=last,
                    tile_position=(0, 0),
                    skip_group_check=True,
                )
                nc.tensor.matmul(
                    out=pv_t[64:97, 0:NQ],
                    lhsT=v_sb[m][:, 34 * hB : 34 * hB + 33],
                    rhs=p_sb[:, NQ : 2 * NQ],
                    start=first,
                    stop=last,
                    tile_position=(0, 64),
                    skip_group_check=True,
                )
                # lazy kT projections AFTER this tile's pv: their DVE cast
                # then queues behind (not ahead of) the mask-mul, so the
                # exp->mul->pv chain is never delayed.  Results are needed
                # only several tiles later.
                if g2 == 0 and m % 4 == 3 and 7 <= m <= 19:
                    emit_kT(0, 4 + (m - 7) // 4)
                if g2 == 1 and m % 4 == 0:
                    emit_kT(1, m // 4)

            # pair epilogue: copy the raw pv bank to SBUF so the bank frees
            # immediately, and take the denominator reciprocals right away.
            if g2 < 3:
                # DMA round-trip broadcast; its ~5us latency hides under the
                # next pair's loop
                nc.vector.tensor_copy(out=o_raw[g2], in_=pv_t[:, 0:NQ])
                nc.vector.reciprocal_approx_fast(
                    out=rec_sb[:, 512 * g2 : 512 * g2 + NQ],
                    in_=o_raw[g2],
                )
                emit_norm(g2)
            else:
                # tail: PE is idle here, so broadcast the denominator rows
                # with a bf16 ones-pattern matmul (no DRAM round-trip),
                # reciprocal on the positive PSUM broadcast, then normalize.
                # One bf16 cast-copy serves both the bcast rhs and the
                # norm-mul numerator.
                nc.vector.tensor_copy(out=od_bf, in_=pv_t[:, 0:NQ])
                bc_ps = ps_pool.tile([128, 1024], F32, name="bc_ps", tag="s")
                nc.tensor.matmul(
                    out=bc_ps[:, 0:NQ],
                    lhsT=ones_bf,
                    rhs=od_bf,
                    start=True,
                    stop=True,
                )
                nc.vector.reciprocal_approx_fast(
                    out=rec_sb[:, 512 * g2 : 512 * g2 + NQ], in_=bc_ps[:, 0:NQ]
                )
                nc.vector.tensor_mul(
                    out=o_cat[g2],
                    in0=od_bf,
                    in1=rec_sb[:, 512 * g2 : 512 * g2 + NQ],
                )

        # final projection: full K=128 accumulating matmuls against the
        # host-side zero-padded reordering of w_proj.
        for t in range(2):
            f_ps = ps_pool.tile([128, 1024], F32, name="f_ps", tag="s")
            for g2 in range(4):
                nc.tensor.matmul(
                    out=f_ps[:, 0:NQ],
                    lhsT=wp_sb[:, g2, 128 * t : 128 * (t + 1)],
                    rhs=o_cat[g2],
                    start=(g2 == 0),
                    stop=(g2 == 3),
                )
            fin = consts.tile([128, NQ], F32, name=f"fin{t}", tag=f"fin{t}")
            for hf in range(2):
                qs = slice(256 * hf, 256 * (hf + 1))
                nc.scalar.activation(
                    out=fin[:, qs],
                    in_=f_ps[:, qs],
                    func=IDENT,
                    bias=bias_sb[:, t : t + 1],
                    scale=1.0,
                )
                eng = (nc.sync, nc.scalar)[hf]
                eng.dma_start(out=out_d[128 * t : 128 * (t + 1), qs], in_=fin[:, qs])

    nc.compile()
    return nc


def _get_nc():
    if "nc" not in _CACHE:
        _CACHE["nc"] = build_kernel()
    return _CACHE["nc"]


def _prep_inputs(x, adj, w_qkv, w_proj, b_proj):
    x = np.asarray(x, dtype=np.float32).reshape(N, C)
    adj = np.asarray(adj).reshape(N, N)
    w_qkv = np.asarray(w_qkv, dtype=np.float32)
    w_proj = np.asarray(w_proj, dtype=np.float32)
    b_proj = np.asarray(b_proj, dtype=np.float32)

    scale = float(DH) ** -0.5
    wqkvT = w_qkv.T.copy()
    wqkvT[:, 0:C] *= scale  # fold attention scale into q projection
    wqkvT = np.ascontiguousarray(wqkvT, dtype=BF16NP)
    wprojT = w_proj.T.astype(np.float32)
    wproj2 = np.zeros((4 * 128, C), dtype=np.float32)
    for g2 in range(4):
        wproj2[128 * g2 + 0 : 128 * g2 + 32] = wprojT[64 * g2 : 64 * g2 + 32]
        wproj2[128 * g2 + 64 : 128 * g2 + 96] = wprojT[64 * g2 + 32 : 64 * g2 + 64]
    wproj2 = np.ascontiguousarray(wproj2, dtype=BF16NP)
    bias2 = np.ascontiguousarray(b_proj.reshape(2, 128).T, dtype=np.float32)
    xT = np.ascontiguousarray(x.T, dtype=BF16NP)

    keep = (adj > 0).T  # [kpos, qrow]
    one_bf_bits = np.float32(1.0).astype(BF16NP).view(np.uint16).astype(np.int32)
    maskT = np.empty((N, N), dtype=np.int16)
    for m in range(KT):
        rows = slice(128 * m, 128 * (m + 1))
        k = keep[rows]
        if m in V_TILES:
            maskT[rows] = np.where(k, B2I, B2I - MASK_DELTA).astype(np.int16)
        else:
            maskT[rows] = np.where(k, one_bf_bits, 0).astype(np.int16)

    in_maps = []
    for i in range(NCORES):
        sl = slice(NQ * i, NQ * (i + 1))
        m1 = np.ascontiguousarray(maskT[:, sl])
        in_maps.append(
            {
                "xT": xT,
                "xqT": np.ascontiguousarray(xT[:, sl]),
                "wqkv": wqkvT,
                "wproj2": wproj2,
                "bias2": bias2,
                "maskT": np.ascontiguousarray(np.concatenate([m1, m1], axis=1)),
            }
        )
    return in_maps


def run_on_hw(inputs, trace=False):
    from concourse.bass_utils import run_bass_kernel_spmd

    nc = _get_nc()
    in_maps = _prep_inputs(**inputs)
    res = run_bass_kernel_spmd(
        nc, in_maps, core_ids=list(range(NCORES)), trace=trace
    )
    out = np.empty((1, N, C), dtype=np.float32)
    for i in range(NCORES):
        out[0, NQ * i : NQ * (i + 1), :] = res.results[i]["out"].T
    return out, res


def kernel(x, adj, w_qkv, w_proj, b_proj):
    out, _ = run_on_hw(
        {"x": x, "adj": adj, "w_qkv": w_qkv, "w_proj": w_proj, "b_proj": b_proj}
    )
    return out


# revision 97
# speedup vs baseline: 1.1911x; 1.1911x over previous
"""Masked multi-head attention on 8 trn2 NeuronCores (Bass/Tile).

Problem: B=1, N=4096, C=256, H=8 heads (Dh=32), dense 0/1 mask adj
shared across heads.  reference:
    qkv = x @ w_qkv.T ; q,k,v per head
    attn = softmax(where(adj>0, q@k.T*scale, -9e15))
    out  = (attn @ v) @ w_proj.T + b_proj

Sharding: sequence-parallel over query rows.  Core i handles query rows
[512*i, 512*(i+1)) for ALL heads; k/v are recomputed on every core from
the (small) full x.  The big tensor -- the 64MB mask -- is split 8 ways
with no replication and there are no collectives.

Per-core pipeline (bf16 matmuls, f32 PSUM), proven baseline structure
plus three changes:
  * startup DMAs are spread across the SP and Activation HWDGE trigger
    queues (+ Pool SWDGE for bulk) so the first matmul inputs land ~6us
    earlier;
  * the post-exp mask multiply runs as scalar_tensor_tensor
    ((p * 1.0) * mask) instead of tensor_tensor -- InstTensorScalarPtr
    is eligible for the DVE 4x_2p perf mode (tensor_tensor mult caps at
    2x_1p) -- with a separate output tile so all views stay SBUF/bf16;
  * V_TILES k-tiles skip ScalarE entirely: one fused DVE
    scalar_tensor_tensor computes int16 bf16-bits =
    round(s*(128/ln2) + maskbias), a Schraudolph exp with the additive
    mask folded into the int16 bias tile (same bytes as the bf16 mask,
    per-tile format).  Masked lanes get bits ~2^-43.
"""

import sys

for _p in ("/opt/trn_rl_repo", "/root/.axon_site/_ro/trn_rl_repo"):
    if _p not in sys.path:
        sys.path.insert(0, _p)

import numpy as np
import ml_dtypes

BF16NP = ml_dtypes.bfloat16

N = 4096
C = 256
H = 8
DH = 32
NCORES = 8
NQ = N // NCORES  # 512 query rows per core
KT = N // 128  # 32 key tiles

A2 = 128.0 / np.log(2.0)  # Schraudolph scale
B2I = 16249  # 16256 + C2, C2 = -7 (round-to-nearest calibrated)
MASK_DELTA = 5540  # round(30 * A2): masked lanes -> bits ~ 2^-43

# k-tiles whose exp runs as the fused DVE Schraudolph op (int16 mask bias).
# Empty: every attempt to offload exp off ScalarE lost time -- the score
# PSUM banks must be freed by ScalarE, not via the busy in-order DVE queue.
V_TILES = frozenset()

_CACHE = {}


def build_kernel():
    import concourse.bacc as bacc
    import concourse.tile as tile
    from concourse import mybir

    F32 = mybir.dt.float32
    BF = mybir.dt.bfloat16
    I16 = mybir.dt.int16
    EXP = mybir.ActivationFunctionType.Exp
    IDENT = mybir.ActivationFunctionType.Identity

    nc = bacc.Bacc("TRN2", target_bir_lowering=False, debug=False, num_devices=NCORES)

    xT_d = nc.dram_tensor("xT", [C, N], BF, kind="ExternalInput")
    xqT_d = nc.dram_tensor("xqT", [C, NQ], BF, kind="ExternalInput")
    wqkv_d = nc.dram_tensor("wqkv", [C, 3 * C], BF, kind="ExternalInput")
    wproj2_d = nc.dram_tensor("wproj2", [4 * 128, C], BF, kind="ExternalInput")
    bias2_d = nc.dram_tensor("bias2", [128, 2], F32, kind="ExternalInput")
    maskT_d = nc.dram_tensor("maskT", [N, 2 * NQ], I16, kind="ExternalInput")
    out_d = nc.dram_tensor("out", [C, NQ], F32, kind="ExternalOutput")

    with (
        tile.TileContext(nc) as tc,
        tc.tile_pool(name="consts", bufs=1) as consts,
        tc.tile_pool(name="ps", bufs=3, space="PSUM") as ps_pool,
        tc.tile_pool(name="pvps", bufs=2, space="PSUM") as pv_pool,
        tc.tile_pool(name="ptile", bufs=10) as p_pool,
        tc.tile_pool(name="dramp", bufs=1, space="DRAM") as dram_pool,
    ):
        # ---------------- input DMAs, spread across trigger queues ------
        w_sb = [
            consts.tile([128, 3 * C], BF, name=f"w_sb{c}", tag=f"w{c}")
            for c in range(2)
        ]
        xq_sb = [
            consts.tile([128, NQ], BF, name=f"xq_sb{c}", tag=f"xq{c}") for c in range(2)
        ]
        xT_sb = [
            consts.tile([128, N], BF, name=f"xT_sb{c}", tag=f"xT{c}") for c in range(2)
        ]
        mask_sb = consts.tile([128, KT, 2 * NQ], I16)
        mask_bf = mask_sb.bitcast(BF)
        maskT_r = maskT_d.rearrange("(m p) q -> p m q", p=128)

        nc.sync.dma_start(out=w_sb[0], in_=wqkv_d[0:128, :])
        nc.scalar.dma_start(out=w_sb[1], in_=wqkv_d[128:256, :])
        nc.scalar.dma_start(out=xq_sb[0], in_=xqT_d[0:128, :])
        nc.sync.dma_start(out=xq_sb[1], in_=xqT_d[128:256, :])
        nc.scalar.dma_start(out=xT_sb[0][:, 0:1024], in_=xT_d[0:128, 0:1024])
        nc.sync.dma_start(out=xT_sb[1][:, 0:1024], in_=xT_d[128:256, 0:1024])
        nc.sync.dma_start(out=xT_sb[0][:, 1024:2048], in_=xT_d[0:128, 1024:2048])
        nc.scalar.dma_start(out=xT_sb[1][:, 1024:2048], in_=xT_d[128:256, 1024:2048])
        nc.sync.dma_start(out=mask_sb[:, 0:2, :], in_=maskT_r[:, 0:2, :])
        nc.scalar.dma_start(out=mask_sb[:, 2:4, :], in_=maskT_r[:, 2:4, :])
        nc.gpsimd.dma_start(out=mask_sb[:, 4:6, :], in_=maskT_r[:, 4:6, :])
        # bulk tail + end-of-kernel tensors on the SWDGE path.  Mask tiles
        # 6:12 go BEFORE the xT tail: the mask-muls consume tiles at the
        # exp cadence from ~t+20us and a late chunk stalls the whole
        # exp->mul->pv chain once the probs pool fills.
        nc.gpsimd.dma_start(out=mask_sb[:, 6:12, :], in_=maskT_r[:, 6:12, :])
        for ch in range(2, 4):
            cs = slice(ch * 1024, (ch + 1) * 1024)
            for c in range(2):
                nc.gpsimd.dma_start(
                    out=xT_sb[c][:, cs], in_=xT_d[128 * c : 128 * (c + 1), cs]
                )
        nc.gpsimd.dma_start(out=mask_sb[:, 12:22, :], in_=maskT_r[:, 12:22, :])
        nc.gpsimd.dma_start(out=mask_sb[:, 22:32, :], in_=maskT_r[:, 22:32, :])
        wp_sb = consts.tile([128, 4, C], BF)
        nc.gpsimd.dma_start(
            out=wp_sb, in_=wproj2_d.rearrange("(g p) c -> p g c", p=128)
        )
        bias_sb = consts.tile([128, 2], F32)
        nc.gpsimd.dma_start(out=bias_sb, in_=bias2_d[:])

        # ones-pattern lhsT for the last pair's denominator broadcast matmul
        # (out rows 0:64 <- rhs row 32, 64:128 <- row 96); memset FIRST so
        # it doubles as the warmup operand below.
        ones_bf = consts.tile([128, 128], BF)
        nc.vector.memset(ones_bf, 0.0)
        nc.vector.memset(ones_bf[32:33, 0:64], 1.0)
        nc.vector.memset(ones_bf[96:97, 64:128], 1.0)

        # PE p-state warmup: the clock needs ~3us of continuous execution
        # to reach 2.4GHz, and phase-1 otherwise runs at the 1.2GHz mid
        # state.  Burn dummy matmuls on the (early-memset) ones tile while
        # the input DMAs are still in flight.
        warm_ps = ps_pool.tile([128, 1024], F32, name="warm_ps", tag="s")

        def warm(n):
            for _ in range(n):
                nc.tensor.matmul(
                    out=warm_ps[:, 0:128],
                    lhsT=ones_bf,
                    rhs=ones_bf,
                    start=True,
                    stop=True,
                )

        warm(52)  # ramp + bridge the PE to the w-DMA arrival (~8.5us)

        # ---------------- phase 1: qT, kT, v ----------------
        qT_sb = [
            consts.tile([128, NQ], BF, name=f"qT_sb{g}", tag=f"qT{g}") for g in range(2)
        ]
        for g in range(2):
            q_ps = ps_pool.tile([128, 1024], F32, name="q_ps", tag="s")
            for c in range(2):
                nc.tensor.matmul(
                    out=q_ps[:, 0:NQ],
                    lhsT=w_sb[c][:, 128 * g : 128 * (g + 1)],
                    rhs=xq_sb[c],
                    start=(c == 0),
                    stop=(c == 1),
                )
            nc.vector.tensor_copy(out=qT_sb[g], in_=q_ps[:, 0:NQ])

        # keep the clock hot while the PE waits for the xT chunks: these
        # run in-order between q-proj and the first kT projections
        warm(14)

        kT_sb = [
            consts.tile([128, N], BF, name=f"kT_sb{g}", tag=f"kT{g}") for g in range(2)
        ]

        def emit_kT(g, n):
            k_ps = ps_pool.tile([128, 1024], F32, name="k_ps", tag="s")
            for c in range(2):
                nc.tensor.matmul(
                    out=k_ps[:, 0:512],
                    lhsT=w_sb[c][:, 256 + 128 * g : 256 + 128 * (g + 1)],
                    rhs=xT_sb[c][:, 512 * n : 512 * (n + 1)],
                    start=(c == 0),
                    stop=(c == 1),
                )
            nc.vector.tensor_copy(
                out=kT_sb[g][:, 512 * n : 512 * (n + 1)], in_=k_ps[:, 0:512]
            )

        v_sb = [
            consts.tile([128, 34 * H], BF, name=f"v_sb_{m}", tag=f"v{m}")
            for m in range(KT)
        ]

        def emit_v(m):
            v_ps = ps_pool.tile([128, 1024], F32, name="v_ps", tag="s")
            for c in range(2):
                nc.tensor.matmul(
                    out=v_ps[:, 0:C],
                    lhsT=xT_sb[c][:, 128 * m : 128 * (m + 1)],
                    rhs=w_sb[c][:, 512:768],
                    start=(c == 0),
                    stop=(c == 1),
                )
            vt3 = v_sb[m].rearrange("p (h w) -> p h w", h=H)
            nc.gpsimd.memset(vt3[:, :, 32:34], 1.0)
            nc.vector.tensor_copy(
                out=vt3[:, :, 0:32],
                in_=v_ps[:, 0:C].rearrange("p (h w) -> p h w", h=H),
            )

        for n in range(min(4, N // 512)):
            if n == 2:
                # bridge the wait for the second xT kilochunk so the kT
                # projections stay at full clock
                warm(10)
            emit_kT(0, n)

        # ---------------- phase 2: attention ----------------
        rec_sb = consts.tile([128, 4 * 512], F32)
        bc_cat = consts.tile([128, 4 * 512], F32)
        nc.vector.memset(bc_cat, 0.0)
        od_bf = consts.tile([128, NQ], BF)
        o_cat = [
            consts.tile([128, NQ], BF, name=f"o_cat{g2}", tag=f"oc{g2}")
            for g2 in range(4)
        ]
        o_raw = [
            consts.tile([128, NQ], F32, name=f"o_raw{g2}", tag=f"or{g2}")
            for g2 in range(4)
        ]

        nrm_scr = dram_pool.tile([4, 2, NQ], F32)

        def emit_norm(j):
            # broadcast reciprocal rows 32/96 across each head's 32
            # partitions via a DRAM round-trip: DMA the row out, DMA it back
            # with a partition-broadcast source AP (no PE/PSUM involved).
            import concourse.bass as bass

            for half, prow, orow in ((0, 32, 0), (1, 96, 64)):
                nc.sync.dma_start(
                    out=nrm_scr[j, half, :],
                    in_=rec_sb[prow : prow + 1, 512 * j : 512 * j + NQ],
                )
                row = nrm_scr[j, half : half + 1, :]
                bcast = bass.AP(
                    tensor=row.tensor,
                    offset=row.offset,
                    ap=[[0, 32]] + list(row.ap[1:]),
                )
                nc.sync.dma_start(
                    out=bc_cat[orow : orow + 32, 512 * j : 512 * j + NQ],
                    in_=bcast,
                )
            nc.vector.tensor_mul(
                out=o_cat[j],
                in0=o_raw[j],
                in1=bc_cat[:, 512 * j : 512 * j + NQ],
            )

        for g2 in range(4):  # head pairs
            hA, hB = 2 * g2, 2 * g2 + 1
            gA, pA = hA // 4, 32 * (hA % 4)
            gB, pB = hB // 4, 32 * (hB % 4)
            pv_t = pv_pool.tile([128, 512], F32, name="pv_t", tag="pv")
            if g2 < 2 or KT < 32:
                nc.vector.memset(pv_t, 1.0)
            for m in range(KT):
                # v is needed by this very tile's pv -> emit first
                if g2 == 0:
                    emit_v(m)
                s_ps = ps_pool.tile([128, 1024], F32, name="s_ps", tag="s")
                nc.tensor.matmul(
                    out=s_ps[:, 0:NQ],
                    lhsT=kT_sb[gA][pA : pA + 32, 128 * m : 128 * (m + 1)],
                    rhs=qT_sb[gA][pA : pA + 32, :],
                    start=True,
                    stop=True,
                    tile_position=(pA, 0),
                )
                nc.tensor.matmul(
                    out=s_ps[:, 512 : 512 + NQ],
                    lhsT=kT_sb[gB][pB : pB + 32, 128 * m : 128 * (m + 1)],
                    rhs=qT_sb[gB][pB : pB + 32, :],
                    start=True,
                    stop=True,
                    tile_position=(pB, 0),
                )
                p_sb = p_pool.tile([128, 2 * NQ], BF, name="p_sb", tag="p")
                if m in V_TILES:
                    # fused Schraudolph exp + mask in ONE DVE op: int16
                    # bf16-bits = round(s * A2 + maskbias)
                    nc.vector.scalar_tensor_tensor(
                        out=p_sb.bitcast(I16),
                        in0=s_ps,
                        scalar=float(A2),
                        in1=mask_sb[:, m, :],
                        op0=mybir.AluOpType.mult,
                        op1=mybir.AluOpType.add,
                    )
                else:
                    nc.scalar.activation(out=p_sb, in_=s_ps, func=EXP)
                    nc.vector.tensor_mul(out=p_sb, in0=p_sb, in1=mask_bf[:, m, :])
                first, last = (m == 0), (m == KT - 1)
                nc.tensor.matmul(
                    out=pv_t[0:33, 0:NQ],
                    lhsT=v_sb[m][:, 34 * hA : 34 * hA + 33],
                    rhs=p_sb[:, 0:NQ],
                    start=first,
                    stop=last,
                    tile_position=(0, 0),
                    skip_group_check=True,
                )
                nc.tensor.matmul(
                    out=pv_t[64:97, 0:NQ],
                    lhsT=v_sb[m][:, 34 * hB : 34 * hB + 33],
                    rhs=p_sb[:, NQ : 2 * NQ],
                    start=first,
                    stop=last,
                    tile_position=(0, 64),
                    skip_group_check=True,
                )
                # lazy kT projections AFTER this tile's pv: their DVE cast
                # then queues behind (not ahead of) the mask-mul, so the
                # exp->mul->pv chain is never delayed.  Results are needed
                # only several tiles later.
                if g2 == 0 and m % 4 == 3 and 4 + m // 4 < N // 512:
                    emit_kT(0, 4 + m // 4)
                if g2 == 1 and m % 4 == 0:
                    emit_kT(1, m // 4)

            # pair epilogue: copy the raw pv bank to SBUF so the bank frees
            # immediately, and take the denominator reciprocals right away.
            if g2 < 3:
                # DMA round-trip broadcast; its ~5us latency hides under the
                # next pair's loop
                nc.vector.tensor_copy(out=o_raw[g2], in_=pv_t[:, 0:NQ])
                nc.vector.reciprocal_approx_fast(
                    out=rec_sb[:, 512 * g2 : 512 * g2 + NQ],
                    in_=o_raw[g2],
                )
                emit_norm(g2)
            else:
                # tail: PE is idle here, so broadcast the denominator rows
                # with a bf16 ones-pattern matmul (no DRAM round-trip),
                # reciprocal on the positive PSUM broadcast, then normalize.
                # One bf16 cast-copy serves both the bcast rhs and the
                # norm-mul numerator.
                nc.vector.tensor_copy(out=od_bf, in_=pv_t[:, 0:NQ])
                bc_ps = ps_pool.tile([128, 1024], F32, name="bc_ps", tag="s")
                nc.tensor.matmul(
                    out=bc_ps[:, 0:NQ],
                    lhsT=ones_bf,
                    rhs=od_bf,
                    start=True,
                    stop=True,
                )
                nc.vector.reciprocal_approx_fast(
                    out=rec_sb[:, 512 * g2 : 512 * g2 + NQ], in_=bc_ps[:, 0:NQ]
                )
                nc.vector.tensor_mul(
                    out=o_cat[g2],
                    in0=od_bf,
                    in1=rec_sb[:, 512 * g2 : 512 * g2 + NQ],
                )

        # final projection: full K=128 accumulating matmuls against the
        # host-side zero-padded reordering of w_proj.
        for t in range(2):
            f_ps = ps_pool.tile([128, 1024], F32, name="f_ps", tag="s")
            for g2 in range(4):
                nc.tensor.matmul(
                    out=f_ps[:, 0:NQ],
                    lhsT=wp_sb[:, g2, 128 * t : 128 * (t + 1)],
                    rhs=o_cat[g2],
                    start=(g2 == 0),
                    stop=(g2 == 3),
                )
            fin = consts.tile([128, NQ], F32, name=f"fin{t}", tag=f"fin{t}")
            for hf in range(2):
                qs = slice(256 * hf, 256 * (hf + 1))
                nc.scalar.activation(
                    out=fin[:, qs],
                    in_=f_ps[:, qs],
                    func=IDENT,
                    bias=bias_sb[:, t : t + 1],
                    scale=1.0,
                )
                eng = (nc.sync, nc.scalar)[hf]
                eng.dma_start(out=out_d[128 * t : 128 * (t + 1), qs], in_=fin[:, qs])

    nc.compile()
    return nc


def _get_nc():
    if "nc" not in _CACHE:
        _CACHE["nc"] = build_kernel()
    return _CACHE["nc"]


def _prep_inputs(x, adj, w_qkv, w_proj, b_proj):
    x = np.asarray(x, dtype=np.float32).reshape(N, C)
    adj = np.asarray(adj).reshape(N, N)
    w_qkv = np.asarray(w_qkv, dtype=np.float32)
    w_proj = np.asarray(w_proj, dtype=np.float32)
    b_proj = np.asarray(b_proj, dtype=np.float32)

    scale = float(DH) ** -0.5
    wqkvT = w_qkv.T.copy()
    wqkvT[:, 0:C] *= scale  # fold attention scale into q projection
    wqkvT = np.ascontiguousarray(wqkvT, dtype=BF16NP)
    wprojT = w_proj.T.astype(np.float32)
    wproj2 = np.zeros((4 * 128, C), dtype=np.float32)
    for g2 in range(4):
        wproj2[128 * g2 + 0 : 128 * g2 + 32] = wprojT[64 * g2 : 64 * g2 + 32]
        wproj2[128 * g2 + 64 : 128 * g2 + 96] = wprojT[64 * g2 + 32 : 64 * g2 + 64]
    wproj2 = np.ascontiguousarray(wproj2, dtype=BF16NP)
    bias2 = np.ascontiguousarray(b_proj.reshape(2, 128).T, dtype=np.float32)
    xT = np.ascontiguousarray(x.T, dtype=BF16NP)

    keep = (adj > 0).T  # [kpos, qrow]
    one_bf_bits = np.float32(1.0).astype(BF16NP).view(np.uint16).astype(np.int32)
    maskT = np.empty((N, N), dtype=np.int16)
    for m in range(KT):
        rows = slice(128 * m, 128 * (m + 1))
        k = keep[rows]
        if m in V_TILES:
            maskT[rows] = np.where(k, B2I, B2I - MASK_DELTA).astype(np.int16)
        else:
            maskT[rows] = np.where(k, one_bf_bits, 0).astype(np.int16)

    in_maps = []
    for i in range(NCORES):
        sl = slice(NQ * i, NQ * (i + 1))
        m1 = np.ascontiguousarray(maskT[:, sl])
        in_maps.append(
            {
                "xT": xT,
                "xqT": np.ascontiguousarray(xT[:, sl]),
                "wqkv": wqkvT,
                "wproj2": wproj2,
                "bias2": bias2,
                "maskT": np.ascontiguousarray(np.concatenate([m1, m1], axis=1)),
            }
        )
    return in_maps


def run_on_hw(inputs, trace=False):
    from concourse.bass_utils import run_bass_kernel_spmd

    nc = _get_nc()
    in_maps = _prep_inputs(**inputs)
    res = run_bass_kernel_spmd(
        nc, in_maps, core_ids=list(range(NCORES)), trace=trace
    )
    out = np.empty((1, N, C), dtype=np.float32)
    for i in range(NCORES):
        out[0, NQ * i : NQ * (i + 1), :] = res.results[i]["out"].T
    return out, res


def kernel(x, adj, w_qkv, w_proj, b_proj):
    out, _ = run_on_hw(
        {"x": x, "adj": adj, "w_qkv": w_qkv, "w_proj": w_proj, "b_proj": b_proj}
    )
    return out


# revision 99
# speedup vs baseline: 1.1958x; 1.0040x over previous
"""Masked multi-head attention on 8 trn2 NeuronCores (Bass/Tile).

Problem: B=1, N=4096, C=256, H=8 heads (Dh=32), dense 0/1 mask adj
shared across heads.  reference:
    qkv = x @ w_qkv.T ; q,k,v per head
    attn = softmax(where(adj>0, q@k.T*scale, -9e15))
    out  = (attn @ v) @ w_proj.T + b_proj

Sharding: sequence-parallel over query rows.  Core i handles query rows
[512*i, 512*(i+1)) for ALL heads; k/v are recomputed on every core from
the (small) full x.  The big tensor -- the 64MB mask -- is split 8 ways
with no replication and there are no collectives.

Per-core pipeline (bf16 matmuls, f32 PSUM), proven baseline structure
plus three changes:
  * startup DMAs are spread across the SP and Activation HWDGE trigger
    queues (+ Pool SWDGE for bulk) so the first matmul inputs land ~6us
    earlier;
  * the post-exp mask multiply runs as scalar_tensor_tensor
    ((p * 1.0) * mask) instead of tensor_tensor -- InstTensorScalarPtr
    is eligible for the DVE 4x_2p perf mode (tensor_tensor mult caps at
    2x_1p) -- with a separate output tile so all views stay SBUF/bf16;
  * V_TILES k-tiles skip ScalarE entirely: one fused DVE
    scalar_tensor_tensor computes int16 bf16-bits =
    round(s*(128/ln2) + maskbias), a Schraudolph exp with the additive
    mask folded into the int16 bias tile (same bytes as the bf16 mask,
    per-tile format).  Masked lanes get bits ~2^-43.
"""

import sys

for _p in ("/opt/trn_rl_repo", "/root/.axon_site/_ro/trn_rl_repo"):
    if _p not in sys.path:
        sys.path.insert(0, _p)

import numpy as np
import ml_dtypes

BF16NP = ml_dtypes.bfloat16

N = 4096
C = 256
H = 8
DH = 32
NCORES = 8
NQ = N // NCORES  # 512 query rows per core
KT = N // 128  # 32 key tiles

A2 = 128.0 / np.log(2.0)  # Schraudolph scale
B2I = 16249  # 16256 + C2, C2 = -7 (round-to-nearest calibrated)
MASK_DELTA = 5540  # round(30 * A2): masked lanes -> bits ~ 2^-43

# k-tiles whose exp runs as the fused DVE Schraudolph op (int16 mask bias).
# Empty: every attempt to offload exp off ScalarE lost time -- the score
# PSUM banks must be freed by ScalarE, not via the busy in-order DVE queue.
V_TILES = frozenset()

_CACHE = {}


def build_kernel():
    import concourse.bacc as bacc
    import concourse.tile as tile
    from concourse import mybir

    F32 = mybir.dt.float32
    BF = mybir.dt.bfloat16
    I16 = mybir.dt.int16
    EXP = mybir.ActivationFunctionType.Exp
    IDENT = mybir.ActivationFunctionType.Identity

    nc = bacc.Bacc("TRN2", target_bir_lowering=False, debug=False, num_devices=NCORES)

    xT_d = nc.dram_tensor("xT", [C, N], BF, kind="ExternalInput")
    xqT_d = nc.dram_tensor("xqT", [C, NQ], BF, kind="ExternalInput")
    wqkv_d = nc.dram_tensor("wqkv", [C, 3 * C], BF, kind="ExternalInput")
    wproj2_d = nc.dram_tensor("wproj2", [4 * 128, C], BF, kind="ExternalInput")
    bias2_d = nc.dram_tensor("bias2", [128, 2], F32, kind="ExternalInput")
    maskT_d = nc.dram_tensor("maskT", [N, 2 * NQ], I16, kind="ExternalInput")
    out_d = nc.dram_tensor("out", [C, NQ], F32, kind="ExternalOutput")

    with (
        tile.TileContext(nc) as tc,
        tc.tile_pool(name="consts", bufs=1) as consts,
        tc.tile_pool(name="ps", bufs=3, space="PSUM") as ps_pool,
        tc.tile_pool(name="pvps", bufs=2, space="PSUM") as pv_pool,
        tc.tile_pool(name="ptile", bufs=10) as p_pool,
        tc.tile_pool(name="dramp", bufs=1, space="DRAM") as dram_pool,
    ):
        # ---------------- input DMAs, spread across trigger queues ------
        w_sb = [
            consts.tile([128, 3 * C], BF, name=f"w_sb{c}", tag=f"w{c}")
            for c in range(2)
        ]
        xq_sb = [
            consts.tile([128, NQ], BF, name=f"xq_sb{c}", tag=f"xq{c}") for c in range(2)
        ]
        xT_sb = [
            consts.tile([128, N], BF, name=f"xT_sb{c}", tag=f"xT{c}") for c in range(2)
        ]
        mask_sb = consts.tile([128, KT, 2 * NQ], I16)
        mask_bf = mask_sb.bitcast(BF)
        maskT_r = maskT_d.rearrange("(m p) q -> p m q", p=128)

        nc.sync.dma_start(out=w_sb[0], in_=wqkv_d[0:128, :])
        nc.scalar.dma_start(out=w_sb[1], in_=wqkv_d[128:256, :])
        nc.scalar.dma_start(out=xq_sb[0], in_=xqT_d[0:128, :])
        nc.sync.dma_start(out=xq_sb[1], in_=xqT_d[128:256, :])
        nc.scalar.dma_start(out=xT_sb[0][:, 0:1024], in_=xT_d[0:128, 0:1024])
        nc.sync.dma_start(out=xT_sb[1][:, 0:1024], in_=xT_d[128:256, 0:1024])
        nc.sync.dma_start(out=xT_sb[0][:, 1024:2048], in_=xT_d[0:128, 1024:2048])
        nc.scalar.dma_start(out=xT_sb[1][:, 1024:2048], in_=xT_d[128:256, 1024:2048])
        nc.sync.dma_start(out=mask_sb[:, 0:2, :], in_=maskT_r[:, 0:2, :])
        nc.scalar.dma_start(out=mask_sb[:, 2:4, :], in_=maskT_r[:, 2:4, :])
        # tiles 12:14 ride the otherwise-idle Act queue: on the saturated
        # SWDGE path they land ~3us after their muls want them
        nc.scalar.dma_start(out=mask_sb[:, 12:14, :], in_=maskT_r[:, 12:14, :])
        nc.gpsimd.dma_start(out=mask_sb[:, 4:6, :], in_=maskT_r[:, 4:6, :])
        # bulk tail + end-of-kernel tensors on the SWDGE path.  Mask tiles
        # 6:12 go BEFORE the xT tail: the mask-muls consume tiles at the
        # exp cadence from ~t+20us and a late chunk stalls the whole
        # exp->mul->pv chain once the probs pool fills.
        nc.gpsimd.dma_start(out=mask_sb[:, 6:12, :], in_=maskT_r[:, 6:12, :])
        for ch in range(2, 4):
            cs = slice(ch * 1024, (ch + 1) * 1024)
            for c in range(2):
                nc.gpsimd.dma_start(
                    out=xT_sb[c][:, cs], in_=xT_d[128 * c : 128 * (c + 1), cs]
                )
        nc.gpsimd.dma_start(out=mask_sb[:, 14:22, :], in_=maskT_r[:, 14:22, :])
        nc.gpsimd.dma_start(out=mask_sb[:, 22:32, :], in_=maskT_r[:, 22:32, :])
        wp_sb = consts.tile([128, 4, C], BF)
        nc.gpsimd.dma_start(
            out=wp_sb, in_=wproj2_d.rearrange("(g p) c -> p g c", p=128)
        )
        bias_sb = consts.tile([128, 2], F32)
        nc.gpsimd.dma_start(out=bias_sb, in_=bias2_d[:])

        # ones-pattern lhsT for the last pair's denominator broadcast matmul
        # (out rows 0:64 <- rhs row 32, 64:128 <- row 96); memset FIRST so
        # it doubles as the warmup operand below.
        ones_bf = consts.tile([128, 128], BF)
        nc.vector.memset(ones_bf, 0.0)
        nc.vector.memset(ones_bf[32:33, 0:64], 1.0)
        nc.vector.memset(ones_bf[96:97, 64:128], 1.0)

        # PE p-state warmup: the clock needs ~3us of continuous execution
        # to reach 2.4GHz, and phase-1 otherwise runs at the 1.2GHz mid
        # state.  Burn dummy matmuls on the (early-memset) ones tile while
        # the input DMAs are still in flight.
        warm_ps = ps_pool.tile([128, 1024], F32, name="warm_ps", tag="s")

        def warm(n):
            for _ in range(n):
                nc.tensor.matmul(
                    out=warm_ps[:, 0:128],
                    lhsT=ones_bf,
                    rhs=ones_bf,
                    start=True,
                    stop=True,
                )

        warm(52)  # ramp + bridge the PE to the w-DMA arrival (~8.5us)

        # ---------------- phase 1: qT, kT, v ----------------
        qT_sb = [
            consts.tile([128, NQ], BF, name=f"qT_sb{g}", tag=f"qT{g}") for g in range(2)
        ]
        for g in range(2):
            q_ps = ps_pool.tile([128, 1024], F32, name="q_ps", tag="s")
            for c in range(2):
                nc.tensor.matmul(
                    out=q_ps[:, 0:NQ],
                    lhsT=w_sb[c][:, 128 * g : 128 * (g + 1)],
                    rhs=xq_sb[c],
                    start=(c == 0),
                    stop=(c == 1),
                )
            nc.vector.tensor_copy(out=qT_sb[g], in_=q_ps[:, 0:NQ])

        # keep the clock hot while the PE waits for the xT chunks: these
        # run in-order between q-proj and the first kT projections
        warm(14)

        kT_sb = [
            consts.tile([128, N], BF, name=f"kT_sb{g}", tag=f"kT{g}") for g in range(2)
        ]

        def emit_kT(g, n):
            k_ps = ps_pool.tile([128, 1024], F32, name="k_ps", tag="s")
            for c in range(2):
                nc.tensor.matmul(
                    out=k_ps[:, 0:512],
                    lhsT=w_sb[c][:, 256 + 128 * g : 256 + 128 * (g + 1)],
                    rhs=xT_sb[c][:, 512 * n : 512 * (n + 1)],
                    start=(c == 0),
                    stop=(c == 1),
                )
            nc.vector.tensor_copy(
                out=kT_sb[g][:, 512 * n : 512 * (n + 1)], in_=k_ps[:, 0:512]
            )

        v_sb = [
            consts.tile([128, 34 * H], BF, name=f"v_sb_{m}", tag=f"v{m}")
            for m in range(KT)
        ]

        def emit_v(m):
            v_ps = ps_pool.tile([128, 1024], F32, name="v_ps", tag="s")
            for c in range(2):
                nc.tensor.matmul(
                    out=v_ps[:, 0:C],
                    lhsT=xT_sb[c][:, 128 * m : 128 * (m + 1)],
                    rhs=w_sb[c][:, 512:768],
                    start=(c == 0),
                    stop=(c == 1),
                )
            vt3 = v_sb[m].rearrange("p (h w) -> p h w", h=H)
            nc.gpsimd.memset(vt3[:, :, 32:34], 1.0)
            nc.vector.tensor_copy(
                out=vt3[:, :, 0:32],
                in_=v_ps[:, 0:C].rearrange("p (h w) -> p h w", h=H),
            )

        for n in range(min(4, N // 512)):
            if n == 2:
                # bridge the wait for the second xT kilochunk so the kT
                # projections stay at full clock
                warm(10)
            emit_kT(0, n)

        # ---------------- phase 2: attention ----------------
        rec_sb = consts.tile([128, 4 * 512], F32)
        bc_cat = consts.tile([128, 4 * 512], F32)
        nc.vector.memset(bc_cat, 0.0)
        od_bf = consts.tile([128, NQ], BF)
        o_cat = [
            consts.tile([128, NQ], BF, name=f"o_cat{g2}", tag=f"oc{g2}")
            for g2 in range(4)
        ]
        o_raw = [
            consts.tile([128, NQ], F32, name=f"o_raw{g2}", tag=f"or{g2}")
            for g2 in range(4)
        ]

        nrm_scr = dram_pool.tile([4, 2, NQ], F32)

        def emit_norm(j):
            # broadcast reciprocal rows 32/96 across each head's 32
            # partitions via a DRAM round-trip: DMA the row out, DMA it back
            # with a partition-broadcast source AP (no PE/PSUM involved).
            import concourse.bass as bass

            for half, prow, orow in ((0, 32, 0), (1, 96, 64)):
                nc.sync.dma_start(
                    out=nrm_scr[j, half, :],
                    in_=rec_sb[prow : prow + 1, 512 * j : 512 * j + NQ],
                )
                row = nrm_scr[j, half : half + 1, :]
                bcast = bass.AP(
                    tensor=row.tensor,
                    offset=row.offset,
                    ap=[[0, 32]] + list(row.ap[1:]),
                )
                nc.sync.dma_start(
                    out=bc_cat[orow : orow + 32, 512 * j : 512 * j + NQ],
                    in_=bcast,
                )
            nc.vector.tensor_mul(
                out=o_cat[j],
                in0=o_raw[j],
                in1=bc_cat[:, 512 * j : 512 * j + NQ],
            )

        for g2 in range(4):  # head pairs
            hA, hB = 2 * g2, 2 * g2 + 1
            gA, pA = hA // 4, 32 * (hA % 4)
            gB, pB = hB // 4, 32 * (hB % 4)
            pv_t = pv_pool.tile([128, 512], F32, name="pv_t", tag="pv")
            if g2 < 2 or KT < 32:
                nc.vector.memset(pv_t, 1.0)
            for m in range(KT):
                # v is needed by this very tile's pv -> emit first
                if g2 == 0:
                    emit_v(m)
                s_ps = ps_pool.tile([128, 1024], F32, name="s_ps", tag="s")
                nc.tensor.matmul(
                    out=s_ps[:, 0:NQ],
                    lhsT=kT_sb[gA][pA : pA + 32, 128 * m : 128 * (m + 1)],
                    rhs=qT_sb[gA][pA : pA + 32, :],
                    start=True,
                    stop=True,
                    tile_position=(pA, 0),
                )
                nc.tensor.matmul(
                    out=s_ps[:, 512 : 512 + NQ],
                    lhsT=kT_sb[gB][pB : pB + 32, 128 * m : 128 * (m + 1)],
                    rhs=qT_sb[gB][pB : pB + 32, :],
                    start=True,
                    stop=True,
                    tile_position=(pB, 0),
                )
                p_sb = p_pool.tile([128, 2 * NQ], BF, name="p_sb", tag="p")
                if m in V_TILES:
                    # fused Schraudolph exp + mask in ONE DVE op: int16
                    # bf16-bits = round(s * A2 + maskbias)
                    nc.vector.scalar_tensor_tensor(
                        out=p_sb.bitcast(I16),
                        in0=s_ps,
                        scalar=float(A2),
                        in1=mask_sb[:, m, :],
                        op0=mybir.AluOpType.mult,
                        op1=mybir.AluOpType.add,
                    )
                else:
                    nc.scalar.activation(out=p_sb, in_=s_ps, func=EXP)
                    nc.vector.tensor_mul(out=p_sb, in0=p_sb, in1=mask_bf[:, m, :])
                first, last = (m == 0), (m == KT - 1)
                nc.tensor.matmul(
                    out=pv_t[0:33, 0:NQ],
                    lhsT=v_sb[m][:, 34 * hA : 34 * hA + 33],
                    rhs=p_sb[:, 0:NQ],
                    start=first,
                    stop=last,
                    tile_position=(0, 0),
                    skip_group_check=True,
                )
                nc.tensor.matmul(
                    out=pv_t[64:97, 0:NQ],
                    lhsT=v_sb[m][:, 34 * hB : 34 * hB + 33],
                    rhs=p_sb[:, NQ : 2 * NQ],
                    start=first,
                    stop=last,
                    tile_position=(0, 64),
                    skip_group_check=True,
                )
                # lazy kT projections AFTER this tile's pv: their DVE cast
                # then queues behind (not ahead of) the mask-mul, so the
                # exp->mul->pv chain is never delayed.  Results are needed
                # only several tiles later.
                if g2 == 0 and m % 4 == 3 and 4 + m // 4 < N // 512:
                    emit_kT(0, 4 + m // 4)
                if g2 == 1 and m % 4 == 0:
                    emit_kT(1, m // 4)

            # pair epilogue: copy the raw pv bank to SBUF so the bank frees
            # immediately, and take the denominator reciprocals right away.
            if g2 < 3:
                # DMA round-trip broadcast; its ~5us latency hides under the
                # next pair's loop
                nc.vector.tensor_copy(out=o_raw[g2], in_=pv_t[:, 0:NQ])
                nc.vector.reciprocal_approx_fast(
                    out=rec_sb[:, 512 * g2 : 512 * g2 + NQ],
                    in_=o_raw[g2],
                )
                emit_norm(g2)
            else:
                # tail: PE is idle here, so broadcast the denominator rows
                # with a bf16 ones-pattern matmul (no DRAM round-trip),
                # reciprocal on the positive PSUM broadcast, then normalize.
                # One bf16 cast-copy serves both the bcast rhs and the
                # norm-mul numerator.
                nc.vector.tensor_copy(out=od_bf, in_=pv_t[:, 0:NQ])
                bc_ps = ps_pool.tile([128, 1024], F32, name="bc_ps", tag="s")
                nc.tensor.matmul(
                    out=bc_ps[:, 0:NQ],
                    lhsT=ones_bf,
                    rhs=od_bf,
                    start=True,
                    stop=True,
                )
                nc.vector.reciprocal_approx_fast(
                    out=rec_sb[:, 512 * g2 : 512 * g2 + NQ], in_=bc_ps[:, 0:NQ]
                )
                nc.vector.tensor_mul(
                    out=o_cat[g2],
                    in0=od_bf,
                    in1=rec_sb[:, 512 * g2 : 512 * g2 + NQ],
                )

        # final projection: full K=128 accumulating matmuls against the
        # host-side zero-padded reordering of w_proj.
        for t in range(2):
            f_ps = ps_pool.tile([128, 1024], F32, name="f_ps", tag="s")
            for g2 in range(4):
                nc.tensor.matmul(
                    out=f_ps[:, 0:NQ],
                    lhsT=wp_sb[:, g2, 128 * t : 128 * (t + 1)],
                    rhs=o_cat[g2],
                    start=(g2 == 0),
                    stop=(g2 == 3),
                )
            fin = consts.tile([128, NQ], F32, name=f"fin{t}", tag=f"fin{t}")
            for hf in range(2):
                qs = slice(256 * hf, 256 * (hf + 1))
                nc.scalar.activation(
                    out=fin[:, qs],
                    in_=f_ps[:, qs],
                    func=IDENT,
                    bias=bias_sb[:, t : t + 1],
                    scale=1.0,
                )
                eng = (nc.sync, nc.scalar)[hf]
                eng.dma_start(out=out_d[128 * t : 128 * (t + 1), qs], in_=fin[:, qs])

    nc.compile()
    return nc


def _get_nc():
    if "nc" not in _CACHE:
        _CACHE["nc"] = build_kernel()
    return _CACHE["nc"]


def _prep_inputs(x, adj, w_qkv, w_proj, b_proj):
    x = np.asarray(x, dtype=np.float32).reshape(N, C)
    adj = np.asarray(adj).reshape(N, N)
    w_qkv = np.asarray(w_qkv, dtype=np.float32)
    w_proj = np.asarray(w_proj, dtype=np.float32)
    b_proj = np.asarray(b_proj, dtype=np.float32)

    scale = float(DH) ** -0.5
    wqkvT = w_qkv.T.copy()
    wqkvT[:, 0:C] *= scale  # fold attention scale into q projection
    wqkvT = np.ascontiguousarray(wqkvT, dtype=BF16NP)
    wprojT = w_proj.T.astype(np.float32)
    wproj2 = np.zeros((4 * 128, C), dtype=np.float32)
    for g2 in range(4):
        wproj2[128 * g2 + 0 : 128 * g2 + 32] = wprojT[64 * g2 : 64 * g2 + 32]
        wproj2[128 * g2 + 64 : 128 * g2 + 96] = wprojT[64 * g2 + 32 : 64 * g2 + 64]
    wproj2 = np.ascontiguousarray(wproj2, dtype=BF16NP)
    bias2 = np.ascontiguousarray(b_proj.reshape(2, 128).T, dtype=np.float32)
    xT = np.ascontiguousarray(x.T, dtype=BF16NP)

    keep = (adj > 0).T  # [kpos, qrow]
    one_bf_bits = np.float32(1.0).astype(BF16NP).view(np.uint16).astype(np.int32)
    maskT = np.empty((N, N), dtype=np.int16)
    for m in range(KT):
        rows = slice(128 * m, 128 * (m + 1))
        k = keep[rows]
        if m in V_TILES:
            maskT[rows] = np.where(k, B2I, B2I - MASK_DELTA).astype(np.int16)
        else:
            maskT[rows] = np.where(k, one_bf_bits, 0).astype(np.int16)

    in_maps = []
    for i in range(NCORES):
        sl = slice(NQ * i, NQ * (i + 1))
        m1 = np.ascontiguousarray(maskT[:, sl])
        in_maps.append(
            {
                "xT": xT,
                "xqT": np.ascontiguousarray(xT[:, sl]),
                "wqkv": wqkvT,
                "wproj2": wproj2,
                "bias2": bias2,
                "maskT": np.ascontiguousarray(np.concatenate([m1, m1], axis=1)),
            }
        )
    return in_maps


def run_on_hw(inputs, trace=False):
    from concourse.bass_utils import run_bass_kernel_spmd

    nc = _get_nc()
    in_maps = _prep_inputs(**inputs)
    res = run_bass_kernel_spmd(
        nc, in_maps, core_ids=list(range(NCORES)), trace=trace
    )
    out = np.empty((1, N, C), dtype=np.float32)
    for i in range(NCORES):
        out[0, NQ * i : NQ * (i + 1), :] = res.results[i]["out"].T
    return out, res


def kernel(x, adj, w_qkv, w_proj, b_proj):
    out, _ = run_on_hw(
        {"x": x, "adj": adj, "w_qkv": w_qkv, "w_proj": w_proj, "b_proj": b_proj}
    )
    return out
